# revision 35
# baseline (speedup 1.0000x reference)
"""Bass/Trainium2 kernel for nn_DotProductAttention (B=32, Q=K=1024, D=512).

Strategy: data-parallel over batch (4 slots per core x 8 cores), with
mask-aware work skipping. Positions k >= valid_len have softmax weight
exactly 0 (exp(-1e6) underflows), so k-tiles that are fully masked can
be skipped in every matmul. The projection is folded onto the keys side:

  scores = (Q @ W^T) @ K^T = Q @ (K @ W)^T

so the projection work (KW) also shrinks with the mask.

The program is specialized at build time to the actual valid_lens:
batches are sorted by active-k-tile count (desc) and grouped into 4
slots of 8 (one batch per core per slot); each slot's tile count is the
group max (provably optimal for a shared SPMD instruction stream). Tiles
between a batch's own active count and the slot max process real (but
masked) key data — mask bias makes their exp exactly 0.

Per slot (m = slot k-tile count, Ks = 128*m):
  kwT[d,k]   = W-tiles.T @ kT            (contract e; active k only)
  scoresT[k,q] = kwT-tiles.T @ qT        (contract d)
  expT[k,q]  = exp(scoresT/sqrt(d) + maskbias[k])
  denom[q]   = ones.T @ sum_t expT       (bf16 matmul; fp32 is 4x slower)
  out[q,v]   = (expT-slices.T @ values) * (1/denom[q])

The PE stream is software-pipelined across slots: kw of slot s+1 is
emitted between scores_s and out_s, covering the exp-activation tail
and the denominator round-trip so the PE never idles (idle gaps also
drop the PE clock to half speed for ~3us — the p-state ramp).

Softmax max-subtraction is dropped: scores/sqrt(d) ~ N(0,1), exp cannot
overflow. All matmuls in bf16 with fp32 PSUM. Outputs stored bf16
(tolerance absorbs the rounding), halving output DMA.
"""

import numpy as np
import ml_dtypes

import concourse.bass as bass
import concourse.mybir as mybir
from concourse import tile
from concourse.bacc import Bacc
from concourse.bass_utils import run_bass_kernel_spmd

BF16 = mybir.dt.bfloat16
F32 = mybir.dt.float32
AF = mybir.ActivationFunctionType

B, Q, K, D = 32, 1024, 1024, 512
N_CORES = 8
N_SLOTS = B // N_CORES
SCALE = 1.0 / float(np.sqrt(D))
MASK_VALUE = -1000000.0

ET, DT = D // 128, D // 128       # 4 feature tiles of 128
KT = K // 128                     # 8 key tiles of 128 (max)
QT = Q // 128                     # 8 query tiles of 128
QC = Q // 512                     # 2 query chunks of 512 (psum bank limit)


def plan_slots(valid_lens):
    """Sort batches desc by active k-tiles, group into N_SLOTS groups of
    N_CORES. assign[s][c] = batch id; M[s] = group max tile count."""
    vl = np.asarray(valid_lens).astype(np.int64)
    kt = np.ceil(vl / 128).astype(np.int64)
    order = np.argsort(-kt, kind="stable")
    assign = order.reshape(N_SLOTS, N_CORES)
    M = [int(kt[assign[s]].max()) for s in range(N_SLOTS)]
    return assign, M


def build_program(M) -> bass.Bass:
    nc = Bacc()

    slots = [(s, m) for s, m in enumerate(M) if m > 0]
    w_d = nc.dram_tensor("w", (128, ET * D), BF16, kind="ExternalInput")
    qT_d, kT_d, v_d, l_d, mb_d, ov_d, ol_d = {}, {}, {}, {}, {}, {}, {}
    for s, m in slots:
        Ks = 128 * m
        qT_d[s] = nc.dram_tensor(f"qT{s}", (128, DT, Q), BF16, kind="ExternalInput")
        kT_d[s] = nc.dram_tensor(f"kT{s}", (128, ET, Ks), BF16, kind="ExternalInput")
        v_d[s] = nc.dram_tensor(f"v{s}", (128, m, D), BF16, kind="ExternalInput")
        l_d[s] = nc.dram_tensor(f"l{s}", (128, m, D), BF16, kind="ExternalInput")
        mb_d[s] = nc.dram_tensor(f"mb{s}", (128, m), F32, kind="ExternalInput")
        ov_d[s] = nc.dram_tensor(f"ov{s}", (Q, D), BF16, kind="ExternalOutput")
        ol_d[s] = nc.dram_tensor(f"ol{s}", (Q, D), BF16, kind="ExternalOutput")

    with tile.TileContext(nc) as tc:
        with (
            tc.tile_pool(name="wpool", bufs=1) as wpool,
            tc.tile_pool(name="inpool", bufs=2) as inpool,
            tc.tile_pool(name="workpool", bufs=2) as workpool,
            tc.tile_pool(name="outpool", bufs=2) as outpool,
            tc.tile_pool(name="ps_acc", bufs=4, space="PSUM") as ps_acc,
            tc.tile_pool(name="ps_out", bufs=4, space="PSUM") as ps_out,
        ):
            w_sb = wpool.tile([128, ET, D], BF16, tag="w")
            nc.sync.dma_start(w_sb[:], w_d[:])
            ones_f32 = wpool.tile([128, 1], F32, tag="ones_f32")
            nc.vector.memset(ones_f32[:], 1.0)
            ones_bf = wpool.tile([128, 1], BF16, tag="ones_bf")
            nc.vector.memset(ones_bf[:], 1.0)

            # warm the PE HAM clock-gate during the initial input DMAs:
            # ~4us of dummy matmuls flips the clock 1.2 -> 2.4 GHz before
            # the first real matmul issues
            warm_sb = wpool.tile([128, 512], BF16, tag="warm")
            nc.vector.memset(warm_sb[:], 0.0)
            ps_warm = ps_acc.tile([128, 512], F32, tag="ps_acc")
            for _ in range(8):
                nc.tensor.matmul(
                    ps_warm[:], warm_sb[:, 0:128], warm_sb[:], start=True, stop=True
                )

            def chunk_bounds(Ks, first_small):
                # slot 0 leads with a 256-col chunk so kw starts sooner
                bounds, c = [0], 256 if first_small else 512
                while bounds[-1] < Ks:
                    bounds.append(min(bounds[-1] + c, Ks))
                    c = 512
                return bounds

            # per-slot SBUF tiles, created lazily by emit_dma
            sb = {}

            def emit_dma(si):
                s, m = slots[si]
                Ks = 128 * m
                qt_sb = inpool.tile([128, DT, Q], BF16, tag="qt")
                kt_sb = inpool.tile([128, ET, K], BF16, tag="kt")
                v_sb = inpool.tile([128, KT, D], BF16, tag="v")
                l_sb = inpool.tile([128, KT, D], BF16, tag="l")
                mb_sb = workpool.tile([128, KT], F32, tag="mb")
                # single contiguous DMA per tensor: column-chunking made each
                # partition line 4 strided ~1KB segments; unchunked lines are
                # one contiguous 2*ET*Ks-byte read (better descriptor geometry)
                bounds = chunk_bounds(Ks, si == 0)
                nc.sync.dma_start(kt_sb[:, :, :Ks], kT_d[s][:])
                nc.sync.dma_start(qt_sb[:], qT_d[s][:])
                # bounce maskbias onto the ACT engine so downstream exp
                # activations wait on same-engine program order, not a DMA sem
                mb_raw = workpool.tile([128, KT], F32, tag="mb_raw")
                nc.sync.dma_start(mb_raw[:, :m], mb_d[s][:])
                nc.scalar.copy(mb_sb[:, :m], mb_raw[:, :m])
                nc.sync.dma_start(v_sb[:, :m, :], v_d[s][:])
                nc.sync.dma_start(l_sb[:, :m, :], l_d[s][:])
                sb[si] = (qt_sb, kt_sb, v_sb, l_sb, mb_sb, bounds)

            def emit_kw(si):
                # kwT[d,k] = (K @ W).T over active k only
                s, m = slots[si]
                _, kt_sb, _, _, _, bounds = sb[si]
                kw_sb = workpool.tile([128, DT, K], BF16, tag="kw")
                for dt in range(DT):
                    for c0, c1 in zip(bounds, bounds[1:]):
                        cw = c1 - c0
                        ps = ps_acc.tile([128, 512], F32, tag="ps_acc")
                        for et in range(ET):
                            nc.tensor.matmul(
                                ps[:, :cw],
                                w_sb[:, et, dt * 128 : (dt + 1) * 128],
                                kt_sb[:, et, c0:c1],
                                start=(et == 0),
                                stop=(et == ET - 1),
                            )
                        nc.scalar.copy(kw_sb[:, dt, c0:c1], ps[:, :cw])
                sb[si] += (kw_sb,)

            def emit_scores(si):
                # scoresT[k,q] -> expT = exp(scores*SCALE + maskbias[k]);
                # denom partial sums (dacc on DVE) interleave with the loop
                s, m = slots[si]
                qt_sb, _, _, _, mb_sb, _, kw_sb = sb[si]
                exp_sb = workpool.tile([128, KT, Q], BF16, tag="exp")
                dacc = workpool.tile([128, Q], F32, tag="dacc")
                for t in range(m):
                    for qc in range(QC):
                        ps = ps_acc.tile([128, 512], F32, tag="ps_acc")
                        for dt in range(DT):
                            nc.tensor.matmul(
                                ps[:],
                                kw_sb[:, dt, t * 128 : (t + 1) * 128],
                                qt_sb[:, dt, qc * 512 : (qc + 1) * 512],
                                start=(dt == 0),
                                stop=(dt == DT - 1),
                            )
                        nc.scalar.activation(
                            exp_sb[:, t, qc * 512 : (qc + 1) * 512],
                            ps[:],
                            AF.Exp,
                            bias=mb_sb[:, t : t + 1],
                            scale=SCALE,
                        )
                    if t == 1:
                        nc.vector.tensor_add(
                            dacc[:], exp_sb[:, 0, :], exp_sb[:, 1, :]
                        )
                    elif t >= 2:
                        nc.vector.tensor_add(dacc[:], dacc[:], exp_sb[:, t, :])
                sb[si] += (exp_sb, dacc)

            def emit_den(si):
                # den[q-tile] = dacc-slice.T @ ones: one ap_size-1 matmul per
                # qt puts the denominator q-on-partitions directly (no DRAM
                # round-trip — a DRAM RAW between DMA queues is untracked and
                # raced nondeterministically)
                s, m = slots[si]
                exp_sb, dacc = sb[si][7], sb[si][8]
                if m >= 2:
                    den_src, ones_src = dacc[:], ones_f32
                else:
                    den_src, ones_src = exp_sb[:, 0, :], ones_bf
                rcol = workpool.tile([128, QT], F32, tag="rcol")
                for qt in range(QT):
                    psd = ps_acc.tile([128, 1], F32, tag="ps_acc")
                    nc.tensor.matmul(
                        psd[:],
                        den_src[:, qt * 128 : (qt + 1) * 128],
                        ones_src[:],
                        start=True,
                        stop=True,
                    )
                    nc.vector.reciprocal(rcol[:, qt : qt + 1], psd[:])
                sb[si] += (rcol,)

            def emit_out(si):
                # out[q,v] = (expT.T @ values) * (1/denom[q]), drained per qt
                s, m = slots[si]
                _, _, v_sb, l_sb, _, _, _, exp_sb, _, rcol = sb[si]
                ov_stage = outpool.tile([128, QT, D], BF16, tag="ov_stage")
                ol_stage = outpool.tile([128, QT, D], BF16, tag="ol_stage")
                for qt in range(QT):
                    psv = ps_out.tile([128, 512], F32, tag="ps_out")
                    psl = ps_out.tile([128, 512], F32, tag="ps_out")
                    for t in range(m):
                        lhs = exp_sb[:, t, qt * 128 : (qt + 1) * 128]
                        nc.tensor.matmul(
                            psv[:], lhs, v_sb[:, t, :],
                            start=(t == 0), stop=(t == m - 1),
                        )
                        nc.tensor.matmul(
                            psl[:], lhs, l_sb[:, t, :],
                            start=(t == 0), stop=(t == m - 1),
                        )
                    nc.vector.tensor_scalar_mul(
                        ov_stage[:, qt, :], psv[:], rcol[:, qt : qt + 1]
                    )
                    nc.vector.tensor_scalar_mul(
                        ol_stage[:, qt, :], psl[:], rcol[:, qt : qt + 1]
                    )
                    sl = slice(qt * 128, (qt + 1) * 128)
                    nc.sync.dma_start(ov_d[s][sl, :], ov_stage[:, qt, :])
                    nc.sync.dma_start(ol_d[s][sl, :], ol_stage[:, qt, :])

            emit_dma(0)
            emit_kw(0)
            for si in range(len(slots)):
                emit_scores(si)
                if si + 1 < len(slots):
                    emit_dma(si + 1)
                    emit_kw(si + 1)
                emit_den(si)
                emit_out(si)

    nc.finalize()
    # NOTE: an LDWEIGHTS-dedup pass (reuse stationary operand across paired
    # matmuls) was tried here and produced wrong results on HW. Do not re-add.
    return nc


def make_in_maps(queries, keys, values, labels, W, valid_lens, assign, M):
    """Host-side shard + layout prep. All numpy, fp32 -> bf16 casts.
    All tensors are pre-tiled to the SBUF layout (128 partitions first)
    so every input DMA is a plain strided copy."""
    bf = ml_dtypes.bfloat16
    q32 = np.asarray(queries, np.float32)
    k32 = np.asarray(keys, np.float32)
    v32 = np.asarray(values, np.float32)
    l32 = np.asarray(labels, np.float32)
    w32 = np.asarray(W, np.float32)
    vl = np.asarray(valid_lens).astype(np.int64)

    # w_sb[p, et*D + d] = W[et*128 + p, d]  (e on partitions, 128-tiled)
    w_pe = np.ascontiguousarray(
        w32.reshape(ET, 128, D).transpose(1, 0, 2).reshape(128, ET * D)
    ).astype(bf)

    in_maps = []
    for c in range(N_CORES):
        im = {"w": w_pe}
        for s, m in enumerate(M):
            if m == 0:
                continue
            Ks = 128 * m
            b = int(assign[s][c])
            im[f"qT{s}"] = np.ascontiguousarray(
                q32[b].T.reshape(DT, 128, Q).transpose(1, 0, 2)
            ).astype(bf)
            im[f"kT{s}"] = np.ascontiguousarray(
                k32[b, :Ks, :].T.reshape(ET, 128, Ks).transpose(1, 0, 2)
            ).astype(bf)
            im[f"v{s}"] = np.ascontiguousarray(
                v32[b, :Ks, :].reshape(m, 128, D).transpose(1, 0, 2)
            ).astype(bf)
            im[f"l{s}"] = np.ascontiguousarray(
                l32[b, :Ks, :].reshape(m, 128, D).transpose(1, 0, 2)
            ).astype(bf)
            # maskbias[p, t] = 0 if (t*128+p) < valid_len else MASK_VALUE
            mb = np.where(np.arange(Ks) < vl[b], 0.0, MASK_VALUE).astype(
                np.float32
            )
            im[f"mb{s}"] = np.ascontiguousarray(mb.reshape(m, 128).T)
        in_maps.append(im)
    return in_maps


def _fixup_all_masked(out_v, out_l, values, labels, valid_lens):
    """valid_len==0 -> reference softmax is uniform over ALL positions."""
    vl = np.asarray(valid_lens).astype(np.int64)
    for b in np.nonzero(vl == 0)[0]:
        out_v[b, :, :] = np.asarray(values[b], np.float32).mean(axis=0)[None, :]
        out_l[b, :, :] = np.asarray(labels[b], np.float32).mean(axis=0)[None, :]
    return out_v, out_l


def run(queries, keys, values, labels, W, valid_lens, trace=False):
    assign, M = plan_slots(valid_lens)
    if max(M) == 0:
        out_v = np.zeros((B, Q, D), np.float32)
        out_l = np.zeros((B, Q, D), np.float32)
        out_v, out_l = _fixup_all_masked(out_v, out_l, values, labels, valid_lens)
        return (out_v, out_l), None
    nc = build_program(M)
    in_maps = make_in_maps(queries, keys, values, labels, W, valid_lens, assign, M)
    res = run_bass_kernel_spmd(nc, in_maps, list(range(N_CORES)), trace=trace)
    out_v = np.empty((B, Q, D), np.float32)
    out_l = np.empty((B, Q, D), np.float32)
    for s, m in enumerate(M):
        for c in range(N_CORES):
            b = int(assign[s][c])
            if m == 0:
                out_v[b] = 0.0
                out_l[b] = 0.0
            else:
                out_v[b] = res.results[c][f"ov{s}"].astype(np.float32)
                out_l[b] = res.results[c][f"ol{s}"].astype(np.float32)
    out_v, out_l = _fixup_all_masked(out_v, out_l, values, labels, valid_lens)
    return (out_v, out_l), res


def kernel(queries, keys, values, labels, W, valid_lens):
    (out_v, out_l), _ = run(queries, keys, values, labels, W, valid_lens, trace=False)
    return (out_v, out_l)


# revision 36
# speedup vs baseline: 1.0047x; 1.0047x over previous
"""Bass/Trainium2 kernel for nn_DotProductAttention (B=32, Q=K=1024, D=512).

Strategy: data-parallel over batch (4 slots per core x 8 cores), with
mask-aware work skipping. Positions k >= valid_len have softmax weight
exactly 0 (exp(-1e6) underflows), so k-tiles that are fully masked can
be skipped in every matmul. The projection is folded onto the keys side:

  scores = (Q @ W^T) @ K^T = Q @ (K @ W)^T

so the projection work (KW) also shrinks with the mask.

The program is specialized at build time to the actual valid_lens:
batches are sorted by active-k-tile count (desc) and grouped into 4
slots of 8 (one batch per core per slot); each slot's tile count is the
group max (provably optimal for a shared SPMD instruction stream). Tiles
between a batch's own active count and the slot max process real (but
masked) key data — mask bias makes their exp exactly 0.

Per slot (m = slot k-tile count, Ks = 128*m):
  kwT[d,k]   = W-tiles.T @ kT            (contract e; active k only)
  scoresT[k,q] = kwT-tiles.T @ qT        (contract d)
  expT[k,q]  = exp(scoresT/sqrt(d) + maskbias[k])
  denom[q]   = ones.T @ sum_t expT       (bf16 matmul; fp32 is 4x slower)
  out[q,v]   = (expT-slices.T @ values) * (1/denom[q])

The PE stream is software-pipelined across slots: kw of slot s+1 is
emitted between scores_s and out_s, covering the exp-activation tail
and the denominator round-trip so the PE never idles (idle gaps also
drop the PE clock to half speed for ~3us — the p-state ramp).

Softmax max-subtraction is dropped: scores/sqrt(d) ~ N(0,1), exp cannot
overflow. All matmuls in bf16 with fp32 PSUM. Outputs stored bf16
(tolerance absorbs the rounding), halving output DMA.
"""

import numpy as np
import ml_dtypes

import concourse.bass as bass
import concourse.mybir as mybir
from concourse import tile
from concourse.bacc import Bacc
from concourse.bass_utils import run_bass_kernel_spmd

BF16 = mybir.dt.bfloat16
F32 = mybir.dt.float32
AF = mybir.ActivationFunctionType

B, Q, K, D = 32, 1024, 1024, 512
N_CORES = 8
N_SLOTS = B // N_CORES
SCALE = 1.0 / float(np.sqrt(D))
MASK_VALUE = -1000000.0

ET, DT = D // 128, D // 128       # 4 feature tiles of 128
KT = K // 128                     # 8 key tiles of 128 (max)
QT = Q // 128                     # 8 query tiles of 128
QC = Q // 512                     # 2 query chunks of 512 (psum bank limit)


def plan_slots(valid_lens):
    """Sort batches desc by active k-tiles, group into N_SLOTS groups of
    N_CORES. assign[s][c] = batch id; M[s] = group max tile count."""
    vl = np.asarray(valid_lens).astype(np.int64)
    kt = np.ceil(vl / 128).astype(np.int64)
    order = np.argsort(-kt, kind="stable")
    assign = order.reshape(N_SLOTS, N_CORES)
    M = [int(kt[assign[s]].max()) for s in range(N_SLOTS)]
    return assign, M


def build_program(M) -> bass.Bass:
    nc = Bacc()

    slots = [(s, m) for s, m in enumerate(M) if m > 0]
    w_d = nc.dram_tensor("w", (128, ET * D), BF16, kind="ExternalInput")
    qT_d, kT_d, v_d, l_d, mb_d, ov_d, ol_d = {}, {}, {}, {}, {}, {}, {}
    for s, m in slots:
        Ks = 128 * m
        qT_d[s] = nc.dram_tensor(f"qT{s}", (128, DT, Q), BF16, kind="ExternalInput")
        kT_d[s] = nc.dram_tensor(f"kT{s}", (128, ET, Ks), BF16, kind="ExternalInput")
        v_d[s] = nc.dram_tensor(f"v{s}", (128, m, D), BF16, kind="ExternalInput")
        l_d[s] = nc.dram_tensor(f"l{s}", (128, m, D), BF16, kind="ExternalInput")
        mb_d[s] = nc.dram_tensor(f"mb{s}", (128, m), F32, kind="ExternalInput")
        ov_d[s] = nc.dram_tensor(f"ov{s}", (Q, D), BF16, kind="ExternalOutput")
        ol_d[s] = nc.dram_tensor(f"ol{s}", (Q, D), BF16, kind="ExternalOutput")

    with tile.TileContext(nc) as tc:
        with (
            tc.tile_pool(name="wpool", bufs=1) as wpool,
            tc.tile_pool(name="inpool", bufs=2) as inpool,
            tc.tile_pool(name="workpool", bufs=2) as workpool,
            tc.tile_pool(name="outpool", bufs=2) as outpool,
            tc.tile_pool(name="ps_acc", bufs=4, space="PSUM") as ps_acc,
            tc.tile_pool(name="ps_out", bufs=4, space="PSUM") as ps_out,
        ):
            w_sb = wpool.tile([128, ET, D], BF16, tag="w")
            nc.sync.dma_start(w_sb[:], w_d[:])
            ones_f32 = wpool.tile([128, 1], F32, tag="ones_f32")
            nc.vector.memset(ones_f32[:], 1.0)
            ones_bf = wpool.tile([128, 1], BF16, tag="ones_bf")
            nc.vector.memset(ones_bf[:], 1.0)

            # warm the PE HAM clock-gate during the initial input DMAs:
            # ~4us of dummy matmuls flips the clock 1.2 -> 2.4 GHz before
            # the first real matmul issues
            warm_sb = wpool.tile([128, 512], BF16, tag="warm")
            nc.vector.memset(warm_sb[:], 0.0)
            ps_warm = ps_acc.tile([128, 512], F32, tag="ps_acc")
            for _ in range(8):
                nc.tensor.matmul(
                    ps_warm[:], warm_sb[:, 0:128], warm_sb[:], start=True, stop=True
                )

            def chunk_bounds(Ks, first_small):
                # slot 0 leads with a 256-col chunk so kw starts sooner
                bounds, c = [0], 256 if first_small else 512
                while bounds[-1] < Ks:
                    bounds.append(min(bounds[-1] + c, Ks))
                    c = 512
                return bounds

            # per-slot SBUF tiles, created lazily by emit_dma
            sb = {}

            def emit_dma(si):
                s, m = slots[si]
                Ks = 128 * m
                qt_sb = inpool.tile([128, DT, Q], BF16, tag="qt")
                kt_sb = inpool.tile([128, ET, K], BF16, tag="kt")
                v_sb = inpool.tile([128, KT, D], BF16, tag="v")
                l_sb = inpool.tile([128, KT, D], BF16, tag="l")
                mb_sb = workpool.tile([128, KT], F32, tag="mb")
                # single contiguous DMA per tensor: column-chunking made each
                # partition line 4 strided ~1KB segments; unchunked lines are
                # one contiguous 2*ET*Ks-byte read (better descriptor geometry)
                bounds = chunk_bounds(Ks, si == 0)
                nc.sync.dma_start(kt_sb[:, :, :Ks], kT_d[s][:])
                nc.sync.dma_start(qt_sb[:], qT_d[s][:])
                # bounce maskbias onto the ACT engine so downstream exp
                # activations wait on same-engine program order, not a DMA sem
                mb_raw = workpool.tile([128, KT], F32, tag="mb_raw")
                nc.sync.dma_start(mb_raw[:, :m], mb_d[s][:])
                nc.scalar.copy(mb_sb[:, :m], mb_raw[:, :m])
                nc.sync.dma_start(v_sb[:, :m, :], v_d[s][:])
                nc.sync.dma_start(l_sb[:, :m, :], l_d[s][:])
                sb[si] = (qt_sb, kt_sb, v_sb, l_sb, mb_sb, bounds)

            def emit_kw(si):
                # kwT[d,k] = (K @ W).T over active k only
                s, m = slots[si]
                _, kt_sb, _, _, _, bounds = sb[si]
                kw_sb = workpool.tile([128, DT, K], BF16, tag="kw")
                for dt in range(DT):
                    for c0, c1 in zip(bounds, bounds[1:]):
                        cw = c1 - c0
                        ps = ps_acc.tile([128, 512], F32, tag="ps_acc")
                        for et in range(ET):
                            nc.tensor.matmul(
                                ps[:, :cw],
                                w_sb[:, et, dt * 128 : (dt + 1) * 128],
                                kt_sb[:, et, c0:c1],
                                start=(et == 0),
                                stop=(et == ET - 1),
                            )
                        nc.scalar.copy(kw_sb[:, dt, c0:c1], ps[:, :cw])
                sb[si] += (kw_sb,)

            def emit_scores(si):
                # scoresT[k,q] -> expT = exp(scores*SCALE + maskbias[k]);
                # denom partial sums (dacc on DVE) interleave with the loop
                s, m = slots[si]
                qt_sb, _, _, _, mb_sb, _, kw_sb = sb[si]
                exp_sb = workpool.tile([128, KT, Q], BF16, tag="exp")
                dacc = workpool.tile([128, Q], F32, tag="dacc")
                for t in range(m):
                    for qc in range(QC):
                        ps = ps_acc.tile([128, 512], F32, tag="ps_acc")
                        for dt in range(DT):
                            nc.tensor.matmul(
                                ps[:],
                                kw_sb[:, dt, t * 128 : (t + 1) * 128],
                                qt_sb[:, dt, qc * 512 : (qc + 1) * 512],
                                start=(dt == 0),
                                stop=(dt == DT - 1),
                            )
                        nc.scalar.activation(
                            exp_sb[:, t, qc * 512 : (qc + 1) * 512],
                            ps[:],
                            AF.Exp,
                            bias=mb_sb[:, t : t + 1],
                            scale=SCALE,
                        )
                    if t == 1:
                        nc.vector.tensor_add(
                            dacc[:], exp_sb[:, 0, :], exp_sb[:, 1, :]
                        )
                    elif t >= 2:
                        nc.vector.tensor_add(dacc[:], dacc[:], exp_sb[:, t, :])
                sb[si] += (exp_sb, dacc)

            def emit_den(si):
                # den[q-tile] = dacc-slice.T @ ones: one ap_size-1 matmul per
                # qt puts the denominator q-on-partitions directly (no DRAM
                # round-trip — a DRAM RAW between DMA queues is untracked and
                # raced nondeterministically)
                s, m = slots[si]
                exp_sb, dacc = sb[si][7], sb[si][8]
                if m >= 2:
                    den_src, ones_src = dacc[:], ones_f32
                else:
                    den_src, ones_src = exp_sb[:, 0, :], ones_bf
                rcol = workpool.tile([128, QT], F32, tag="rcol")
                for qt in range(QT):
                    psd = ps_acc.tile([128, 1], F32, tag="ps_acc")
                    nc.tensor.matmul(
                        psd[:],
                        den_src[:, qt * 128 : (qt + 1) * 128],
                        ones_src[:],
                        start=True,
                        stop=True,
                    )
                    nc.vector.reciprocal(rcol[:, qt : qt + 1], psd[:])
                sb[si] += (rcol,)

            def emit_out(si):
                # out[q,v] = (expT.T @ values) * (1/denom[q]), drained per qt
                s, m = slots[si]
                _, _, v_sb, l_sb, _, _, _, exp_sb, _, rcol = sb[si]
                ov_stage = outpool.tile([128, QT, D], BF16, tag="ov_stage")
                ol_stage = outpool.tile([128, QT, D], BF16, tag="ol_stage")
                for qt in range(QT):
                    psv = ps_out.tile([128, 512], F32, tag="ps_out")
                    psl = ps_out.tile([128, 512], F32, tag="ps_out")
                    for t in range(m):
                        lhs = exp_sb[:, t, qt * 128 : (qt + 1) * 128]
                        nc.tensor.matmul(
                            psv[:], lhs, v_sb[:, t, :],
                            start=(t == 0), stop=(t == m - 1),
                        )
                        nc.tensor.matmul(
                            psl[:], lhs, l_sb[:, t, :],
                            start=(t == 0), stop=(t == m - 1),
                        )
                    nc.vector.tensor_scalar_mul(
                        ov_stage[:, qt, :], psv[:], rcol[:, qt : qt + 1]
                    )
                    if si == len(slots) - 1:
                        # final slot: ACT is idle (no next-slot exp), so the
                        # psl scaling runs parallel to the DVE's psv scaling —
                        # at m=3 the serial DVE pair rate-matched the PE and
                        # stretched the tail chain
                        nc.scalar.mul(
                            ol_stage[:, qt, :], psl[:], rcol[:, qt : qt + 1]
                        )
                    else:
                        nc.vector.tensor_scalar_mul(
                            ol_stage[:, qt, :], psl[:], rcol[:, qt : qt + 1]
                        )
                    sl = slice(qt * 128, (qt + 1) * 128)
                    nc.sync.dma_start(ov_d[s][sl, :], ov_stage[:, qt, :])
                    nc.sync.dma_start(ol_d[s][sl, :], ol_stage[:, qt, :])

            emit_dma(0)
            emit_kw(0)
            for si in range(len(slots)):
                emit_scores(si)
                if si + 1 < len(slots):
                    emit_dma(si + 1)
                    emit_kw(si + 1)
                emit_den(si)
                emit_out(si)

    nc.finalize()
    # NOTE: an LDWEIGHTS-dedup pass (reuse stationary operand across paired
    # matmuls) was tried here and produced wrong results on HW. Do not re-add.
    return nc


def make_in_maps(queries, keys, values, labels, W, valid_lens, assign, M):
    """Host-side shard + layout prep. All numpy, fp32 -> bf16 casts.
    All tensors are pre-tiled to the SBUF layout (128 partitions first)
    so every input DMA is a plain strided copy."""
    bf = ml_dtypes.bfloat16
    q32 = np.asarray(queries, np.float32)
    k32 = np.asarray(keys, np.float32)
    v32 = np.asarray(values, np.float32)
    l32 = np.asarray(labels, np.float32)
    w32 = np.asarray(W, np.float32)
    vl = np.asarray(valid_lens).astype(np.int64)

    # w_sb[p, et*D + d] = W[et*128 + p, d]  (e on partitions, 128-tiled)
    w_pe = np.ascontiguousarray(
        w32.reshape(ET, 128, D).transpose(1, 0, 2).reshape(128, ET * D)
    ).astype(bf)

    in_maps = []
    for c in range(N_CORES):
        im = {"w": w_pe}
        for s, m in enumerate(M):
            if m == 0:
                continue
            Ks = 128 * m
            b = int(assign[s][c])
            im[f"qT{s}"] = np.ascontiguousarray(
                q32[b].T.reshape(DT, 128, Q).transpose(1, 0, 2)
            ).astype(bf)
            im[f"kT{s}"] = np.ascontiguousarray(
                k32[b, :Ks, :].T.reshape(ET, 128, Ks).transpose(1, 0, 2)
            ).astype(bf)
            im[f"v{s}"] = np.ascontiguousarray(
                v32[b, :Ks, :].reshape(m, 128, D).transpose(1, 0, 2)
            ).astype(bf)
            im[f"l{s}"] = np.ascontiguousarray(
                l32[b, :Ks, :].reshape(m, 128, D).transpose(1, 0, 2)
            ).astype(bf)
            # maskbias[p, t] = 0 if (t*128+p) < valid_len else MASK_VALUE
            mb = np.where(np.arange(Ks) < vl[b], 0.0, MASK_VALUE).astype(
                np.float32
            )
            im[f"mb{s}"] = np.ascontiguousarray(mb.reshape(m, 128).T)
        in_maps.append(im)
    return in_maps


def _fixup_all_masked(out_v, out_l, values, labels, valid_lens):
    """valid_len==0 -> reference softmax is uniform over ALL positions."""
    vl = np.asarray(valid_lens).astype(np.int64)
    for b in np.nonzero(vl == 0)[0]:
        out_v[b, :, :] = np.asarray(values[b], np.float32).mean(axis=0)[None, :]
        out_l[b, :, :] = np.asarray(labels[b], np.float32).mean(axis=0)[None, :]
    return out_v, out_l


def run(queries, keys, values, labels, W, valid_lens, trace=False):
    assign, M = plan_slots(valid_lens)
    if max(M) == 0:
        out_v = np.zeros((B, Q, D), np.float32)
        out_l = np.zeros((B, Q, D), np.float32)
        out_v, out_l = _fixup_all_masked(out_v, out_l, values, labels, valid_lens)
        return (out_v, out_l), None
    nc = build_program(M)
    in_maps = make_in_maps(queries, keys, values, labels, W, valid_lens, assign, M)
    res = run_bass_kernel_spmd(nc, in_maps, list(range(N_CORES)), trace=trace)
    out_v = np.empty((B, Q, D), np.float32)
    out_l = np.empty((B, Q, D), np.float32)
    for s, m in enumerate(M):
        for c in range(N_CORES):
            b = int(assign[s][c])
            if m == 0:
                out_v[b] = 0.0
                out_l[b] = 0.0
            else:
                out_v[b] = res.results[c][f"ov{s}"].astype(np.float32)
                out_l[b] = res.results[c][f"ol{s}"].astype(np.float32)
    out_v, out_l = _fixup_all_masked(out_v, out_l, values, labels, valid_lens)
    return (out_v, out_l), res


def kernel(queries, keys, values, labels, W, valid_lens):
    (out_v, out_l), _ = run(queries, keys, values, labels, W, valid_lens, trace=False)
    return (out_v, out_l)
